# revision 6
# baseline (speedup 1.0000x reference)
"""Trainium2 Bass kernel for nn_DCModule_25451976196444.

Sliding-window (3x3, stride 2) min/max-|anchor-comp| selection pooling:
for each window, pick comp value where |anchor-comp| is minimal and where
it is maximal; output = sum of the two, broadcast over the window footprint
(last-writer-wins => each output pixel maps to its last covering window).

Structure (per core, rows sharded across 8 cores with 1-row halo):
  - load even-row plane (E0) and odd-row plane (O) of each input slab
  - horizontal pass per plane: merge the 3 column candidates per window
    (strict-compare running select => exact first-occurrence tie semantics,
    matching jnp.argmax over the row-major-flattened patch)
  - partition-shift the E0 horizontal results by one row via SBUF->SBUF DMA
    to obtain the third vertical candidate (row 2i+2)
  - vertical pass: merge the 3 row candidates
  - sum min/max selections, duplicate columns on-chip, duplicate rows via
    two strided DMA stores.
Boundary columns/rows (uncovered pixels + duplicated-thrice edges) are fixed
up on the host, which also handles the "clone" semantics of the last row/col.
"""

import numpy as np
from contextlib import ExitStack

import concourse.bass as bass
import concourse.mybir as mybir
import concourse.tile as tile
from concourse import bacc
from concourse import bass_utils
from concourse._compat import with_exitstack

F32 = mybir.dt.float32
I32 = mybir.dt.int32
ALU = mybir.AluOpType
ACTF = mybir.ActivationFunctionType

H = 4096
W = 4096
WS = 3
ST = 2
NCORES = 8
OUTR = H // NCORES          # 512 output rows per core
SLAB = OUTR + 1             # 513 input rows per core (1-row halo)
VR = OUTR // 2              # 256 window-rows per core
NJ_TOT = (W - WS) // ST + 1  # 2047 window-cols
VBLK = 127                  # valid window-rows per block (E0 plane has 128)
NJT = 512                   # window-cols per column tile


def _emit(ctx: ExitStack, tc, a, p, n, outp, outn):
    nc = tc.nc

    in_pool = ctx.enter_context(tc.tile_pool(name="in", bufs=2))
    x_pool = ctx.enter_context(tc.tile_pool(name="x", bufs=2))
    d_pool = ctx.enter_context(tc.tile_pool(name="d", bufs=3))
    m_pool = ctx.enter_context(tc.tile_pool(name="m", bufs=4))
    h_pool = ctx.enter_context(tc.tile_pool(name="h", bufs=2))
    v_pool = ctx.enter_context(tc.tile_pool(name="v", bufs=2))
    o_pool = ctx.enter_context(tc.tile_pool(name="o", bufs=2))

    # column tiles
    jtiles = []
    j0 = 0
    while j0 < NJ_TOT:
        nj = min(NJT, NJ_TOT - j0)
        jtiles.append((j0, nj))
        j0 += nj

    # row blocks
    blocks = []
    i0 = 0
    while i0 < VR:
        nb = min(VBLK, VR - i0)
        blocks.append((i0, nb))
        i0 += nb

    def hpass(dpl, cpl, bp, nj, tag):
        """Horizontal merge over the 3 column candidates of each window.

        dpl: [bp, cw] |anchor - comp|; cpl: [bp, cw] comp values.
        Returns hdM, hcM, hdm, hcm: [bp, nj] selected |diff| and comp values
        for the max and min selectors.  Candidate order v=0,1,2 with strict
        compares so the first occurrence wins on exact ties.
        """
        s0 = slice(0, 2 * nj - 1, 2)
        s1 = slice(1, 2 * nj, 2)
        s2 = slice(2, 2 * nj + 1, 2)
        res = []
        for gt, ext, sel in ((ALU.is_gt, ALU.max, "M"), (ALU.is_lt, ALU.min, "m")):
            mk = m_pool.tile([bp, nj], I32, tag="mk")
            hd1 = d_pool.tile([bp, nj], F32, tag="hd1")
            hc = h_pool.tile([bp, nj], F32, tag=f"hc{tag}{sel}")
            hd = h_pool.tile([bp, nj], F32, tag=f"hd{tag}{sel}")
            nc.vector.tensor_tensor(mk[:], dpl[:, s1], dpl[:, s0], op=gt)
            nc.vector.tensor_tensor(hd1[:], dpl[:, s0], dpl[:, s1], op=ext)
            nc.scalar.copy(hc[:], cpl[:, s0])
            nc.vector.copy_predicated(hc[:], mk[:], cpl[:, s1])
            mk2 = m_pool.tile([bp, nj], I32, tag="mk")
            nc.vector.tensor_tensor(mk2[:], dpl[:, s2], hd1[:], op=gt)
            nc.vector.tensor_tensor(hd[:], hd1[:], dpl[:, s2], op=ext)
            nc.vector.copy_predicated(hc[:], mk2[:], cpl[:, s2])
            res += [hd, hc]
        return res  # hdM, hcM, hdm, hcm

    for (i0, nb) in blocks:
        bpE = nb + 1   # even-row plane rows (need one extra for the shift)
        bpO = nb
        for (j0, nj) in jtiles:
            c0 = 2 * j0
            cw = 2 * nj + 1
            rE = slice(2 * i0, 2 * i0 + 2 * bpE - 1, 2)
            rO = slice(2 * i0 + 1, 2 * i0 + 2 * bpO, 2)
            cs = slice(c0, c0 + cw)

            AE = in_pool.tile([bpE, cw], F32, tag="AE")
            AO = in_pool.tile([bpO, cw], F32, tag="AO")
            PE = in_pool.tile([bpE, cw], F32, tag="PE")
            PO = in_pool.tile([bpO, cw], F32, tag="PO")
            NE = in_pool.tile([bpE, cw], F32, tag="NE")
            NO = in_pool.tile([bpO, cw], F32, tag="NO")
            nc.sync.dma_start(AE[:], a[rE, cs])
            nc.sync.dma_start(AO[:], a[rO, cs])
            nc.sync.dma_start(PE[:], p[rE, cs])
            nc.sync.dma_start(PO[:], p[rO, cs])
            nc.sync.dma_start(NE[:], n[rE, cs])
            nc.sync.dma_start(NO[:], n[rO, cs])

            for CE, CO, OUT, ptag in ((PE, PO, outp, "p"), (NE, NO, outn, "n")):
                xE = x_pool.tile([bpE, cw], F32, tag="xE")
                xO = x_pool.tile([bpO, cw], F32, tag="xO")
                nc.vector.tensor_tensor(xE[:], AE[:], CE[:], op=ALU.subtract)
                nc.vector.tensor_tensor(xO[:], AO[:], CO[:], op=ALU.subtract)
                dE = x_pool.tile([bpE, cw], F32, tag="dE")
                dO = x_pool.tile([bpO, cw], F32, tag="dO")
                nc.scalar.activation(dE[:], xE[:], ACTF.Abs)
                nc.scalar.activation(dO[:], xO[:], ACTF.Abs)

                hdME, hcME, hdmE, hcmE = hpass(dE, CE, bpE, nj, "E")
                hdMO, hcMO, hdmO, hcmO = hpass(dO, CO, bpO, nj, "O")

                # shifted E-plane results: row 2i+2 candidates
                sh = []
                for src, stag in ((hdME, "shdM"), (hcME, "shcM"),
                                  (hdmE, "shdm"), (hcmE, "shcm")):
                    dst = h_pool.tile([nb, nj], F32, tag=stag)
                    nc.sync.dma_start(dst[:], src[1:bpE, :])
                    sh.append(dst)
                hdME1, hcME1, hdmE1, hcmE1 = sh

                # vertical merge: candidates u=0 (E0), u=1 (O), u=2 (E1)
                vcs = []
                for (hdA, hcA, hdB, hcB, hdC, hcC, gt, ext, sel) in (
                    (hdME, hcME, hdMO, hcMO, hdME1, hcME1,
                     ALU.is_gt, ALU.max, "M"),
                    (hdmE, hcmE, hdmO, hcmO, hdmE1, hcmE1,
                     ALU.is_lt, ALU.min, "m"),
                ):
                    mv = m_pool.tile([nb, nj], I32, tag="mk")
                    vd1 = v_pool.tile([nb, nj], F32, tag=f"vd1{sel}")
                    vc = v_pool.tile([nb, nj], F32, tag=f"vc{sel}")
                    nc.vector.tensor_tensor(
                        mv[:], hdB[:bpO], hdA[:bpO], op=gt)
                    nc.vector.tensor_tensor(
                        vd1[:], hdA[:bpO], hdB[:bpO], op=ext)
                    nc.scalar.copy(vc[:], hcA[:nb])
                    nc.vector.copy_predicated(vc[:], mv[:], hcB[:nb])
                    mv2 = m_pool.tile([nb, nj], I32, tag="mk")
                    nc.vector.tensor_tensor(mv2[:], hdC[:], vd1[:nb], op=gt)
                    nc.vector.copy_predicated(vc[:], mv2[:], hcC[:])
                    vcs.append(vc)
                vcM, vcm = vcs

                vv = o_pool.tile([nb, 2 * nj], F32, tag="vv")
                nc.vector.tensor_tensor(
                    vv[:, 0:2 * nj - 1:2], vcm[:], vcM[:], op=ALU.add)
                nc.vector.tensor_tensor(
                    vv[:, 1:2 * nj:2], vcm[:], vcM[:], op=ALU.add)

                ro0 = slice(2 * i0, 2 * i0 + 2 * nb - 1, 2)
                ro1 = slice(2 * i0 + 1, 2 * i0 + 2 * nb, 2)
                co = slice(2 * j0, 2 * j0 + 2 * nj)
                nc.sync.dma_start(OUT[ro0, co], vv[:])
                nc.sync.dma_start(OUT[ro1, co], vv[:])


@with_exitstack
def _tile_kernel(ctx: ExitStack, tc, outs, ins):
    a, p, n = ins
    outp, outn = outs
    _emit(ctx, tc, a, p, n, outp, outn)


_CACHE = {}


def _build():
    if "nc" in _CACHE:
        return _CACHE["nc"]
    nc = bacc.Bacc(
        "TRN2",
        target_bir_lowering=False,
        debug=False,
        enable_asserts=False,
        num_devices=NCORES,
    )
    a = nc.dram_tensor("a", [SLAB, W], F32, kind="ExternalInput").ap()
    p = nc.dram_tensor("p", [SLAB, W], F32, kind="ExternalInput").ap()
    n = nc.dram_tensor("n", [SLAB, W], F32, kind="ExternalInput").ap()
    outp = nc.dram_tensor("outp", [OUTR, W], F32, kind="ExternalOutput").ap()
    outn = nc.dram_tensor("outn", [OUTR, W], F32, kind="ExternalOutput").ap()
    with tile.TileContext(nc) as tc:
        _tile_kernel(tc, [outp, outn], [a, p, n])
    nc.compile()
    _CACHE["nc"] = nc
    return nc


def _make_in_maps(anchor, positive, negative):
    in_maps = []
    for k in range(NCORES):
        r0 = OUTR * k
        m = {}
        for name, t in (("a", anchor), ("p", positive), ("n", negative)):
            s = np.asarray(t[r0:r0 + SLAB], dtype=np.float32)
            if s.shape[0] < SLAB:
                pad = np.zeros((SLAB - s.shape[0], W), np.float32)
                s = np.concatenate([s, pad], axis=0)
            m[name] = np.ascontiguousarray(s)
        in_maps.append(m)
    return in_maps


def _assemble(results, positive, negative):
    pos = np.concatenate([results[k]["outp"] for k in range(NCORES)], axis=0)
    neg = np.concatenate([results[k]["outn"] for k in range(NCORES)], axis=0)
    for out, comp in ((pos, positive), (neg, negative)):
        comp = np.asarray(comp, dtype=np.float32)
        # cols/rows 4094 replicate the last window's value a third time
        out[:, W - 2] = out[:, W - 3]
        out[H - 2, :] = out[H - 3, :]
        # uncovered last row/col keep clone semantics: min-sel + max-sel = 2c
        out[H - 1, :] = 2.0 * comp[H - 1, :]
        out[:, W - 1] = 2.0 * comp[:, W - 1]
    return pos, neg


def run_on_hw(anchor, positive, negative, trace=False):
    nc = _build()
    in_maps = _make_in_maps(anchor, positive, negative)
    res = bass_utils.run_bass_kernel_spmd(
        nc, in_maps, core_ids=list(range(NCORES)), trace=trace)
    pos, neg = _assemble(res.results, positive, negative)
    return (pos, neg), res


def kernel(anchor, positive, negative):
    (pos, neg), _ = run_on_hw(anchor, positive, negative, trace=False)
    return pos, neg


# revision 10
# speedup vs baseline: 2.6891x; 2.6891x over previous
"""Trainium2 Bass kernel for nn_DCModule_25451976196444.

Sliding-window (3x3, stride 2) min/max-|anchor-comp| selection pooling:
for each window, pick the comp value where |anchor-comp| is minimal and
where it is maximal; output = sum of the two, broadcast over the window
footprint (last covering window wins).

Per core (rows sharded across 8 cores):
  - one contiguous 4 MB DMA per input per row-block loads 256 rows as
    [128, 2, 4096] "pair tiles": partition p = image rows (2p, 2p+1); the
    even/odd row planes are contiguous free-dim views
  - horizontal pass per plane merges the 3 column candidates per window
    with strict compares (exact first-occurrence ties, matching the
    row-major flattened argmax/argmin of the reference)
  - the third vertical candidate (row 2i+2) is the even-plane H-result
    shifted by one partition: done on the idle TensorE as a matmul with a
    subdiagonal identity into PSUM (no SBUF-SBUF DMA descriptor storms)
  - vertical pass merges the 3 row candidates; min+max selections are
    summed and column-duplicated on chip
  - row duplication happens in the store DMA via a step-0 source dim; the
    output DRAM layout is column-tile-major so every store is one linear
    transfer (host reassembles)
Each core computes 254 of its 256 window-rows; the host computes the last
2 window-rows per core plus the uncovered boundary rows/cols in numpy with
identical f32 semantics.
"""

import numpy as np
from contextlib import ExitStack

import concourse.bass as bass
import concourse.mybir as mybir
import concourse.tile as tile
from concourse import bacc
from concourse import bass_utils
from concourse._compat import with_exitstack

F32 = mybir.dt.float32
I32 = mybir.dt.int32
ALU = mybir.AluOpType
ACTF = mybir.ActivationFunctionType

H = 4096
W = 4096
WS = 3
ST = 2
NCORES = 8
BP = 128                    # partitions per row-block (pair tiles)
NJT = 512                   # window-cols per column tile


def _geom():
    """(Re)compute derived geometry from H/W/BP/NJT (tests patch these)."""
    global OUTR, SLAB, VR, NJ_TOT, VBLK, JTILES, JOFFS, OUT_ELEMS, BLOCKS
    OUTR = H // NCORES
    SLAB = OUTR
    VR = OUTR // 2
    NJ_TOT = (W - WS) // ST + 1
    VBLK = BP - 1
    assert VR == 2 * VBLK + 2, (VR, VBLK)
    JTILES = []
    j0 = 0
    while j0 < NJ_TOT:
        JTILES.append((j0, min(NJT, NJ_TOT - j0)))
        j0 += NJT
    JOFFS = []
    off = 0
    for (_j, _nj) in JTILES:
        JOFFS.append(off)
        off += OUTR * 2 * _nj
    OUT_ELEMS = off
    BLOCKS = [(0, VBLK), (VBLK, VBLK)]   # device window-rows 0..2*VBLK-1


_geom()


def _emit(ctx: ExitStack, tc, a, p, n, smat, outp, outn):
    nc = tc.nc

    in_pool = ctx.enter_context(tc.tile_pool(name="in", bufs=1))
    x_pool = ctx.enter_context(tc.tile_pool(name="x", bufs=2))
    dd_pool = ctx.enter_context(tc.tile_pool(name="dd", bufs=1))
    t_pool = ctx.enter_context(tc.tile_pool(name="t", bufs=3))
    m_pool = ctx.enter_context(tc.tile_pool(name="m", bufs=3))
    h_pool = ctx.enter_context(tc.tile_pool(name="h", bufs=2))
    v_pool = ctx.enter_context(tc.tile_pool(name="v", bufs=2))
    o_pool = ctx.enter_context(tc.tile_pool(name="o", bufs=2))
    c_pool = ctx.enter_context(tc.tile_pool(name="c", bufs=1))
    ps_pool = ctx.enter_context(tc.tile_pool(name="ps", bufs=1, space="PSUM"))

    sm = c_pool.tile([BP, BP], F32, tag="sm")
    nc.sync.dma_start(sm[:], smat[:])

    def hpass(dpl, cpl, nj, tag):
        """Merge the 3 column candidates of each window along the free dim.

        dpl/cpl: [128, cw] |diff| and comp planes.  Returns hd, hc for the
        max and min selectors; candidate order v=0,1,2 with strict compares
        so the first occurrence wins on exact ties.
        """
        s0 = slice(0, 2 * nj - 1, 2)
        s1 = slice(1, 2 * nj, 2)
        s2 = slice(2, 2 * nj + 1, 2)
        res = []
        for gt, ext, sel in ((ALU.is_gt, ALU.max, "M"), (ALU.is_lt, ALU.min, "m")):
            mk = m_pool.tile([BP, nj], I32, tag="mk")
            hd1 = t_pool.tile([BP, nj], F32, tag="hd1")
            hc = h_pool.tile([BP, nj], F32, tag=f"hc{tag}{sel}")
            hd = h_pool.tile([BP, nj], F32, tag=f"hd{tag}{sel}")
            nc.vector.tensor_tensor(mk[:], dpl[:, s1], dpl[:, s0], op=gt)
            nc.vector.tensor_tensor(hd1[:], dpl[:, s0], dpl[:, s1], op=ext)
            nc.scalar.copy(hc[:], cpl[:, s0])
            nc.vector.copy_predicated(hc[:], mk[:], cpl[:, s1])
            mk2 = m_pool.tile([BP, nj], I32, tag="mk")
            nc.vector.tensor_tensor(mk2[:], dpl[:, s2], hd1[:], op=gt)
            nc.vector.tensor_tensor(hd[:], hd1[:], dpl[:, s2], op=ext)
            nc.vector.copy_predicated(hc[:], mk2[:], cpl[:, s2])
            res += [hd, hc]
        return res  # hdM, hcM, hdm, hcm

    for (i0, nb) in BLOCKS:
        rr = slice(2 * i0, 2 * i0 + 2 * BP)
        AP_ = in_pool.tile([BP, 2, W], F32, tag="A")
        PP_ = in_pool.tile([BP, 2, W], F32, tag="P")
        NP_ = in_pool.tile([BP, 2, W], F32, tag="N")
        nc.sync.dma_start(AP_[:], a[rr, :].rearrange("(q t) w -> q t w", t=2))
        nc.sync.dma_start(PP_[:], p[rr, :].rearrange("(q t) w -> q t w", t=2))
        nc.sync.dma_start(NP_[:], n[rr, :].rearrange("(q t) w -> q t w", t=2))

        for ct, (j0, nj) in enumerate(JTILES):
            c0 = 2 * j0
            cw = 2 * nj + 1
            cs = slice(c0, c0 + cw)
            w = 2 * nj

            for CP_, OUT in ((PP_, outp), (NP_, outn)):
                xp = x_pool.tile([BP, 2, cw], F32, tag="xp")
                dp = dd_pool.tile([BP, 2, cw], F32, tag="dp")
                nc.gpsimd.tensor_tensor(
                    xp[:], AP_[:, :, cs], CP_[:, :, cs], op=ALU.subtract)
                nc.scalar.activation(dp[:], xp[:], ACTF.Abs)

                hdME, hcME, hdmE, hcmE = hpass(
                    dp[:, 0, :], CP_[:, 0, cs], nj, "E")
                hdMO, hcMO, hdmO, hcmO = hpass(
                    dp[:, 1, :], CP_[:, 1, cs], nj, "O")

                # shifted E-plane results (row 2i+2) via TensorE subdiag-
                # identity matmul into PSUM: out[m] = src[m+1], out[127]=0
                sh = []
                for src, stag in ((hdME, "pshdM"), (hcME, "pshcM"),
                                  (hdmE, "pshdm"), (hcmE, "pshcm")):
                    dst = ps_pool.tile([BP, nj], F32, tag=stag)
                    nc.tensor.matmul(
                        dst[:], lhsT=sm[:], rhs=src[:],
                        start=True, stop=True)
                    sh.append(dst)
                hdME1, hcME1, hdmE1, hcmE1 = sh

                # vertical merge: candidates u=0 (E0), u=1 (O), u=2 (E1)
                vcs = []
                for (hdA, hcA, hdB, hcB, hdC, hcC, gt, ext, sel) in (
                    (hdME, hcME, hdMO, hcMO, hdME1, hcME1,
                     ALU.is_gt, ALU.max, "M"),
                    (hdmE, hcmE, hdmO, hcmO, hdmE1, hcmE1,
                     ALU.is_lt, ALU.min, "m"),
                ):
                    mv = m_pool.tile([nb, nj], I32, tag="mk")
                    vd1 = t_pool.tile([nb, nj], F32, tag="hd1")
                    vc = v_pool.tile([nb, nj], F32, tag=f"vc{sel}")
                    nc.vector.tensor_tensor(
                        mv[:], hdB[:nb], hdA[:nb], op=gt)
                    nc.vector.tensor_tensor(
                        vd1[:], hdA[:nb], hdB[:nb], op=ext)
                    nc.scalar.copy(vc[:], hcA[:nb])
                    nc.vector.copy_predicated(vc[:], mv[:], hcB[:nb])
                    mv2 = m_pool.tile([nb, nj], I32, tag="mk")
                    nc.vector.tensor_tensor(mv2[:], hdC[:nb], vd1[:], op=gt)
                    nc.vector.copy_predicated(vc[:], mv2[:], hcC[:nb])
                    vcs.append(vc)
                vcM, vcm = vcs

                vv = o_pool.tile([nb, w], F32, tag="vv")
                nc.gpsimd.tensor_tensor(
                    vv[:, 0:w - 1:2], vcm[:], vcM[:], op=ALU.add)
                nc.gpsimd.tensor_tensor(
                    vv[:, 1:w:2], vcm[:], vcM[:], op=ALU.add)

                # one linear store: rows 2i0..2i0+2nb-1 of this col tile,
                # each vv row written twice via a step-0 source dim
                base = JOFFS[ct] + 2 * i0 * w
                dst = OUT[base:base + 2 * nb * w].rearrange(
                    "(r w) -> r w", w=w)
                src = vv[:, :].unsqueeze(1).broadcast_to([nb, 2, w])
                nc.scalar.dma_start(dst, src)


@with_exitstack
def _tile_kernel(ctx: ExitStack, tc, outs, ins):
    a, p, n, smat = ins
    outp, outn = outs
    _emit(ctx, tc, a, p, n, smat, outp, outn)


_CACHE = {}


def _build():
    if "nc" in _CACHE:
        return _CACHE["nc"]
    nc = bacc.Bacc(
        "TRN2",
        target_bir_lowering=False,
        debug=False,
        enable_asserts=False,
        num_devices=NCORES,
    )
    a = nc.dram_tensor("a", [SLAB, W], F32, kind="ExternalInput").ap()
    p = nc.dram_tensor("p", [SLAB, W], F32, kind="ExternalInput").ap()
    n = nc.dram_tensor("n", [SLAB, W], F32, kind="ExternalInput").ap()
    smat = nc.dram_tensor("s", [BP, BP], F32, kind="ExternalInput").ap()
    outp = nc.dram_tensor("outp", [OUT_ELEMS], F32, kind="ExternalOutput").ap()
    outn = nc.dram_tensor("outn", [OUT_ELEMS], F32, kind="ExternalOutput").ap()
    with tile.TileContext(nc) as tc:
        _tile_kernel(tc, [outp, outn], [a, p, n, smat])
    nc.compile()
    _CACHE["nc"] = nc
    return nc


def _make_in_maps(anchor, positive, negative):
    smat = np.eye(BP, k=-1, dtype=np.float32)
    in_maps = []
    for k in range(NCORES):
        r0 = OUTR * k
        m = {"s": smat}
        for name, t in (("a", anchor), ("p", positive), ("n", negative)):
            m[name] = np.ascontiguousarray(
                np.asarray(t[r0:r0 + SLAB], dtype=np.float32))
        in_maps.append(m)
    return in_maps


def _host_vrow(anchor, comp, r0):
    """Window-row at image rows r0..r0+2, all 2047 col windows; returns the
    min-sel + max-sel comp values [NJ_TOT] with exact reference semantics."""
    a3 = np.asarray(anchor[r0:r0 + 3], dtype=np.float32)
    c3 = np.asarray(comp[r0:r0 + 3], dtype=np.float32)
    d3 = np.abs(a3 - c3)
    dw = np.lib.stride_tricks.sliding_window_view(d3, 3, axis=1)[:, ::2]
    cw_ = np.lib.stride_tricks.sliding_window_view(c3, 3, axis=1)[:, ::2]
    d9 = dw.transpose(1, 0, 2).reshape(NJ_TOT, 9)
    c9 = cw_.transpose(1, 0, 2).reshape(NJ_TOT, 9)
    ar = np.arange(NJ_TOT)
    return c9[ar, np.argmin(d9, axis=1)] + c9[ar, np.argmax(d9, axis=1)]


def _assemble(results, anchor, positive, negative):
    full = {}
    for name, comp in (("outp", positive), ("outn", negative)):
        out = np.zeros((H, W), np.float32)
        for k in range(NCORES):
            flat = results[k][name]
            cols = []
            for ct, (j0, nj) in enumerate(JTILES):
                wct = 2 * nj
                cols.append(
                    flat[JOFFS[ct]:JOFFS[ct] + OUTR * wct].reshape(OUTR, wct))
            out[OUTR * k:OUTR * (k + 1), 0:2 * NJ_TOT] = np.concatenate(
                cols, axis=1)
        # host-computed window-rows: the last 2 per core (device does 254)
        for k in range(NCORES):
            for iv in (2 * VBLK, 2 * VBLK + 1):   # 254, 255
                gi = VR * k + iv
                if 2 * gi + 3 > H:
                    continue   # core 7 last row pair: overwritten below
                vals = np.repeat(_host_vrow(anchor, comp, 2 * gi), 2)
                out[2 * gi, 0:2 * NJ_TOT] = vals
                out[2 * gi + 1, 0:2 * NJ_TOT] = vals
        comp = np.asarray(comp, dtype=np.float32)
        # cols/rows H-2 replicate the last window's value a third time
        out[:, W - 2] = out[:, W - 3]
        out[H - 2, :] = out[H - 3, :]
        # uncovered last row/col keep clone semantics: min-sel + max-sel = 2c
        out[H - 1, :] = 2.0 * comp[H - 1, :]
        out[:, W - 1] = 2.0 * comp[:, W - 1]
        full[name] = out
    return full["outp"], full["outn"]


def run_on_hw(anchor, positive, negative, trace=False):
    nc = _build()
    in_maps = _make_in_maps(anchor, positive, negative)
    res = bass_utils.run_bass_kernel_spmd(
        nc, in_maps, core_ids=list(range(NCORES)), trace=trace)
    pos, neg = _assemble(res.results, anchor, positive, negative)
    return (pos, neg), res


def kernel(anchor, positive, negative):
    (pos, neg), _ = run_on_hw(anchor, positive, negative, trace=False)
    return pos, neg


# revision 11
# speedup vs baseline: 2.7306x; 1.0154x over previous
"""Trainium2 Bass kernel for nn_DCModule_25451976196444.

Sliding-window (3x3, stride 2) min/max-|anchor-comp| selection pooling:
for each window, pick the comp value where |anchor-comp| is minimal and
where it is maximal; output = sum of the two, broadcast over the window
footprint (last covering window wins).

Per core (rows sharded across 8 cores):
  - one contiguous 4 MB DMA per input per row-block loads 256 rows as
    [128, 2, 4096] "pair tiles": partition p = image rows (2p, 2p+1); the
    even/odd row planes are contiguous free-dim views
  - horizontal pass per plane merges the 3 column candidates per window
    with strict compares (exact first-occurrence ties, matching the
    row-major flattened argmax/argmin of the reference)
  - the third vertical candidate (row 2i+2) is the even-plane H-result
    shifted by one partition: done on the idle TensorE as a matmul with a
    subdiagonal identity into PSUM (no SBUF-SBUF DMA descriptor storms)
  - vertical pass merges the 3 row candidates; min+max selections are
    summed and column-duplicated on chip
  - row duplication happens in the store DMA via a step-0 source dim; the
    output DRAM layout is column-tile-major so every store is one linear
    transfer (host reassembles)
Each core computes 254 of its 256 window-rows; the host computes the last
2 window-rows per core plus the uncovered boundary rows/cols in numpy with
identical f32 semantics.
"""

import numpy as np
from contextlib import ExitStack

import concourse.bass as bass
import concourse.mybir as mybir
import concourse.tile as tile
from concourse import bacc
from concourse import bass_utils
from concourse._compat import with_exitstack

F32 = mybir.dt.float32
I32 = mybir.dt.int32
ALU = mybir.AluOpType
ACTF = mybir.ActivationFunctionType

H = 4096
W = 4096
WS = 3
ST = 2
NCORES = 8
BP = 128                    # partitions per row-block (pair tiles)
NJT = 512                   # window-cols per column tile


def _geom():
    """(Re)compute derived geometry from H/W/BP/NJT (tests patch these)."""
    global OUTR, SLAB, VR, NJ_TOT, VBLK, JTILES, JOFFS, OUT_ELEMS, BLOCKS
    OUTR = H // NCORES
    SLAB = OUTR
    VR = OUTR // 2
    NJ_TOT = (W - WS) // ST + 1
    VBLK = BP - 1
    assert VR == 2 * VBLK + 2, (VR, VBLK)
    JTILES = []
    j0 = 0
    while j0 < NJ_TOT:
        JTILES.append((j0, min(NJT, NJ_TOT - j0)))
        j0 += NJT
    JOFFS = []
    off = 0
    for (_j, _nj) in JTILES:
        JOFFS.append(off)
        off += OUTR * 2 * _nj
    OUT_ELEMS = off
    BLOCKS = [(0, VBLK), (VBLK, VBLK)]   # device window-rows 0..2*VBLK-1


_geom()


def _emit(ctx: ExitStack, tc, a, p, n, smat, outp, outn):
    nc = tc.nc

    in_pool = ctx.enter_context(tc.tile_pool(name="in", bufs=1))
    x_pool = ctx.enter_context(tc.tile_pool(name="x", bufs=2))
    dd_pool = ctx.enter_context(tc.tile_pool(name="dd", bufs=1))
    t_pool = ctx.enter_context(tc.tile_pool(name="t", bufs=3))
    m_pool = ctx.enter_context(tc.tile_pool(name="m", bufs=3))
    h_pool = ctx.enter_context(tc.tile_pool(name="h", bufs=2))
    v_pool = ctx.enter_context(tc.tile_pool(name="v", bufs=2))
    o_pool = ctx.enter_context(tc.tile_pool(name="o", bufs=2))
    c_pool = ctx.enter_context(tc.tile_pool(name="c", bufs=1))
    ps_pool = ctx.enter_context(tc.tile_pool(name="ps", bufs=1, space="PSUM"))

    sm = c_pool.tile([BP, BP], F32, tag="sm")
    nc.sync.dma_start(sm[:], smat[:])

    def hpass(dpl, cpl, nj, tag):
        """Merge the 3 column candidates of each window along the free dim.

        dpl/cpl: [128, cw] |diff| and comp planes.  Returns hd, hc for the
        max and min selectors; candidate order v=0,1,2 with strict compares
        so the first occurrence wins on exact ties.
        """
        s0 = slice(0, 2 * nj - 1, 2)
        s1 = slice(1, 2 * nj, 2)
        s2 = slice(2, 2 * nj + 1, 2)
        res = []
        for gt, ext, sel in ((ALU.is_gt, ALU.max, "M"), (ALU.is_lt, ALU.min, "m")):
            mk = m_pool.tile([BP, nj], I32, tag="mk")
            hd1 = t_pool.tile([BP, nj], F32, tag="hd1")
            hc = h_pool.tile([BP, nj], F32, tag=f"hc{tag}{sel}")
            hd = h_pool.tile([BP, nj], F32, tag=f"hd{tag}{sel}")
            nc.vector.tensor_tensor(mk[:], dpl[:, s1], dpl[:, s0], op=gt)
            nc.vector.tensor_tensor(hd1[:], dpl[:, s0], dpl[:, s1], op=ext)
            nc.scalar.copy(hc[:], cpl[:, s0])
            nc.vector.copy_predicated(hc[:], mk[:], cpl[:, s1])
            mk2 = m_pool.tile([BP, nj], I32, tag="mk")
            nc.vector.tensor_tensor(mk2[:], dpl[:, s2], hd1[:], op=gt)
            nc.vector.tensor_tensor(hd[:], hd1[:], dpl[:, s2], op=ext)
            nc.vector.copy_predicated(hc[:], mk2[:], cpl[:, s2])
            res += [hd, hc]
        return res  # hdM, hcM, hdm, hcm

    for (i0, nb) in BLOCKS:
        rr = slice(2 * i0, 2 * i0 + 2 * BP)
        AP_ = in_pool.tile([BP, 2, W], F32, tag="A")
        PP_ = in_pool.tile([BP, 2, W], F32, tag="P")
        NP_ = in_pool.tile([BP, 2, W], F32, tag="N")
        nc.sync.dma_start(AP_[:], a[rr, :].rearrange("(q t) w -> q t w", t=2))
        nc.sync.dma_start(PP_[:], p[rr, :].rearrange("(q t) w -> q t w", t=2))
        nc.sync.dma_start(NP_[:], n[rr, :].rearrange("(q t) w -> q t w", t=2))

        for ct, (j0, nj) in enumerate(JTILES):
            c0 = 2 * j0
            cw = 2 * nj + 1
            cs = slice(c0, c0 + cw)
            w = 2 * nj

            for CP_, OUT in ((PP_, outp), (NP_, outn)):
                xp = x_pool.tile([BP, 2, cw], F32, tag="xp")
                dp = dd_pool.tile([BP, 2, cw], F32, tag="dp")
                nc.gpsimd.tensor_tensor(
                    xp[:], AP_[:, :, cs], CP_[:, :, cs], op=ALU.subtract)
                nc.scalar.activation(dp[:], xp[:], ACTF.Abs)

                hdME, hcME, hdmE, hcmE = hpass(
                    dp[:, 0, :], CP_[:, 0, cs], nj, "E")
                hdMO, hcMO, hdmO, hcmO = hpass(
                    dp[:, 1, :], CP_[:, 1, cs], nj, "O")

                # shifted E-plane results (row 2i+2) via TensorE subdiag-
                # identity matmul into PSUM: out[m] = src[m+1], out[127]=0
                sh = []
                for src, stag in ((hdME, "pshdM"), (hcME, "pshcM"),
                                  (hdmE, "pshdm"), (hcmE, "pshcm")):
                    dst = ps_pool.tile([BP, nj], F32, tag=stag)
                    nc.tensor.matmul(
                        dst[:], lhsT=sm[:], rhs=src[:],
                        start=True, stop=True)
                    sh.append(dst)
                hdME1, hcME1, hdmE1, hcmE1 = sh

                # vertical merge: candidates u=0 (E0), u=1 (O), u=2 (E1)
                vcs = []
                for (hdA, hcA, hdB, hcB, hdC, hcC, gt, ext, sel) in (
                    (hdME, hcME, hdMO, hcMO, hdME1, hcME1,
                     ALU.is_gt, ALU.max, "M"),
                    (hdmE, hcmE, hdmO, hcmO, hdmE1, hcmE1,
                     ALU.is_lt, ALU.min, "m"),
                ):
                    mv = m_pool.tile([nb, nj], I32, tag="mk")
                    vd1 = t_pool.tile([nb, nj], F32, tag="hd1")
                    vc = v_pool.tile([nb, nj], F32, tag=f"vc{sel}")
                    nc.vector.tensor_tensor(
                        mv[:], hdB[:nb], hdA[:nb], op=gt)
                    nc.vector.tensor_tensor(
                        vd1[:], hdA[:nb], hdB[:nb], op=ext)
                    nc.scalar.copy(vc[:], hcA[:nb])
                    nc.vector.copy_predicated(vc[:], mv[:], hcB[:nb])
                    mv2 = m_pool.tile([nb, nj], I32, tag="mk")
                    nc.vector.tensor_tensor(mv2[:], hdC[:nb], vd1[:], op=gt)
                    nc.vector.copy_predicated(vc[:], mv2[:], hcC[:nb])
                    vcs.append(vc)
                vcM, vcm = vcs

                # row-duplicated output tile: free layout [2, w] = the two
                # output rows of each window-row; store is one linear DMA
                # with big per-partition descriptors (spreads across SDMAs)
                vv = o_pool.tile([nb, 2, w], F32, tag="vv")
                nc.vector.tensor_tensor(
                    vv[:, 0, 0:w - 1:2], vcm[:], vcM[:], op=ALU.add)
                nc.vector.tensor_tensor(
                    vv[:, 0, 1:w:2], vcm[:], vcM[:], op=ALU.add)
                nc.scalar.copy(vv[:, 1, :], vv[:, 0, :])

                base = JOFFS[ct] + 2 * i0 * w
                dst = OUT[base:base + 2 * nb * w].rearrange(
                    "(r w) -> r w", w=w)
                nc.sync.dma_start(dst, vv[:])


@with_exitstack
def _tile_kernel(ctx: ExitStack, tc, outs, ins):
    a, p, n, smat = ins
    outp, outn = outs
    _emit(ctx, tc, a, p, n, smat, outp, outn)


_CACHE = {}


def _build():
    if "nc" in _CACHE:
        return _CACHE["nc"]
    nc = bacc.Bacc(
        "TRN2",
        target_bir_lowering=False,
        debug=False,
        enable_asserts=False,
        num_devices=NCORES,
    )
    a = nc.dram_tensor("a", [SLAB, W], F32, kind="ExternalInput").ap()
    p = nc.dram_tensor("p", [SLAB, W], F32, kind="ExternalInput").ap()
    n = nc.dram_tensor("n", [SLAB, W], F32, kind="ExternalInput").ap()
    smat = nc.dram_tensor("s", [BP, BP], F32, kind="ExternalInput").ap()
    outp = nc.dram_tensor("outp", [OUT_ELEMS], F32, kind="ExternalOutput").ap()
    outn = nc.dram_tensor("outn", [OUT_ELEMS], F32, kind="ExternalOutput").ap()
    with tile.TileContext(nc) as tc:
        _tile_kernel(tc, [outp, outn], [a, p, n, smat])
    nc.compile()
    _CACHE["nc"] = nc
    return nc


def _make_in_maps(anchor, positive, negative):
    smat = np.eye(BP, k=-1, dtype=np.float32)
    in_maps = []
    for k in range(NCORES):
        r0 = OUTR * k
        m = {"s": smat}
        for name, t in (("a", anchor), ("p", positive), ("n", negative)):
            m[name] = np.ascontiguousarray(
                np.asarray(t[r0:r0 + SLAB], dtype=np.float32))
        in_maps.append(m)
    return in_maps


def _host_vrow(anchor, comp, r0):
    """Window-row at image rows r0..r0+2, all 2047 col windows; returns the
    min-sel + max-sel comp values [NJ_TOT] with exact reference semantics."""
    a3 = np.asarray(anchor[r0:r0 + 3], dtype=np.float32)
    c3 = np.asarray(comp[r0:r0 + 3], dtype=np.float32)
    d3 = np.abs(a3 - c3)
    dw = np.lib.stride_tricks.sliding_window_view(d3, 3, axis=1)[:, ::2]
    cw_ = np.lib.stride_tricks.sliding_window_view(c3, 3, axis=1)[:, ::2]
    d9 = dw.transpose(1, 0, 2).reshape(NJ_TOT, 9)
    c9 = cw_.transpose(1, 0, 2).reshape(NJ_TOT, 9)
    ar = np.arange(NJ_TOT)
    return c9[ar, np.argmin(d9, axis=1)] + c9[ar, np.argmax(d9, axis=1)]


def _assemble(results, anchor, positive, negative):
    full = {}
    for name, comp in (("outp", positive), ("outn", negative)):
        out = np.zeros((H, W), np.float32)
        for k in range(NCORES):
            flat = results[k][name]
            cols = []
            for ct, (j0, nj) in enumerate(JTILES):
                wct = 2 * nj
                cols.append(
                    flat[JOFFS[ct]:JOFFS[ct] + OUTR * wct].reshape(OUTR, wct))
            out[OUTR * k:OUTR * (k + 1), 0:2 * NJ_TOT] = np.concatenate(
                cols, axis=1)
        # host-computed window-rows: the last 2 per core (device does 254)
        for k in range(NCORES):
            for iv in (2 * VBLK, 2 * VBLK + 1):   # 254, 255
                gi = VR * k + iv
                if 2 * gi + 3 > H:
                    continue   # core 7 last row pair: overwritten below
                vals = np.repeat(_host_vrow(anchor, comp, 2 * gi), 2)
                out[2 * gi, 0:2 * NJ_TOT] = vals
                out[2 * gi + 1, 0:2 * NJ_TOT] = vals
        comp = np.asarray(comp, dtype=np.float32)
        # cols/rows H-2 replicate the last window's value a third time
        out[:, W - 2] = out[:, W - 3]
        out[H - 2, :] = out[H - 3, :]
        # uncovered last row/col keep clone semantics: min-sel + max-sel = 2c
        out[H - 1, :] = 2.0 * comp[H - 1, :]
        out[:, W - 1] = 2.0 * comp[:, W - 1]
        full[name] = out
    return full["outp"], full["outn"]


def run_on_hw(anchor, positive, negative, trace=False):
    nc = _build()
    in_maps = _make_in_maps(anchor, positive, negative)
    res = bass_utils.run_bass_kernel_spmd(
        nc, in_maps, core_ids=list(range(NCORES)), trace=trace)
    pos, neg = _assemble(res.results, anchor, positive, negative)
    return (pos, neg), res


def kernel(anchor, positive, negative):
    (pos, neg), _ = run_on_hw(anchor, positive, negative, trace=False)
    return pos, neg


# revision 12
# speedup vs baseline: 3.3369x; 1.2220x over previous
"""Trainium2 Bass kernel for nn_DCModule_25451976196444.

Sliding-window (3x3, stride 2) min/max-|anchor-comp| selection pooling:
for each window, pick the comp value where |anchor-comp| is minimal and
where it is maximal; output = sum of the two, broadcast over the window
footprint (last covering window wins).

Per core (rows sharded across 8 cores):
  - one contiguous 4 MB DMA per input per row-block loads 256 rows as
    [128, 2, 4096] "pair tiles": partition p = image rows (2p, 2p+1); the
    even/odd row planes are contiguous free-dim views
  - horizontal pass per plane merges the 3 column candidates per window
    with strict compares (exact first-occurrence ties, matching the
    row-major flattened argmax/argmin of the reference)
  - the third vertical candidate (row 2i+2) is the even-plane H-result
    shifted by one partition: done on the idle TensorE as a matmul with a
    subdiagonal identity into PSUM (no SBUF-SBUF DMA descriptor storms)
  - vertical pass merges the 3 row candidates; min+max selections are
    summed and column-duplicated on chip
  - row duplication happens in the store DMA via a step-0 source dim; the
    output DRAM layout is column-tile-major so every store is one linear
    transfer (host reassembles)
Each core computes 254 of its 256 window-rows; the host computes the last
2 window-rows per core plus the uncovered boundary rows/cols in numpy with
identical f32 semantics.
"""

import numpy as np
from contextlib import ExitStack

import concourse.bass as bass
import concourse.mybir as mybir
import concourse.tile as tile
from concourse import bacc
from concourse import bass_utils
from concourse._compat import with_exitstack

F32 = mybir.dt.float32
I32 = mybir.dt.int32
ALU = mybir.AluOpType
ACTF = mybir.ActivationFunctionType

H = 4096
W = 4096
WS = 3
ST = 2
NCORES = 8
BP = 128                    # partitions per row-block (pair tiles)
NJT = 512                   # window-cols per column tile


def _geom():
    """(Re)compute derived geometry from H/W/BP/NJT (tests patch these)."""
    global OUTR, SLAB, VR, NJ_TOT, VBLK, JTILES, JOFFS, OUT_ELEMS, BLOCKS
    OUTR = H // NCORES
    SLAB = OUTR
    VR = OUTR // 2
    NJ_TOT = (W - WS) // ST + 1
    VBLK = BP - 1
    assert VR == 2 * VBLK + 2, (VR, VBLK)
    JTILES = []
    j0 = 0
    while j0 < NJ_TOT:
        JTILES.append((j0, min(NJT, NJ_TOT - j0)))
        j0 += NJT
    JOFFS = []
    off = 0
    for (_j, _nj) in JTILES:
        JOFFS.append(off)
        off += OUTR * 2 * _nj
    OUT_ELEMS = off
    BLOCKS = [(0, VBLK), (VBLK, VBLK)]   # device window-rows 0..2*VBLK-1


_geom()


def _emit(ctx: ExitStack, tc, a, p, n, smat, outp, outn):
    nc = tc.nc

    in_pool = ctx.enter_context(tc.tile_pool(name="in", bufs=1))
    x_pool = ctx.enter_context(tc.tile_pool(name="x", bufs=2))
    dd_pool = ctx.enter_context(tc.tile_pool(name="dd", bufs=1))
    t_pool = ctx.enter_context(tc.tile_pool(name="t", bufs=3))
    m_pool = ctx.enter_context(tc.tile_pool(name="m", bufs=3))
    h_pool = ctx.enter_context(tc.tile_pool(name="h", bufs=2))
    v_pool = ctx.enter_context(tc.tile_pool(name="v", bufs=2))
    o_pool = ctx.enter_context(tc.tile_pool(name="o", bufs=2))
    c_pool = ctx.enter_context(tc.tile_pool(name="c", bufs=1))
    ps_pool = ctx.enter_context(tc.tile_pool(name="ps", bufs=1, space="PSUM"))

    sm = c_pool.tile([BP, BP], F32, tag="sm")
    nc.sync.dma_start(sm[:], smat[:])

    def hpass(dpl, cpl, nj, tag):
        """Merge the 3 column candidates of each window along the free dim.

        dpl/cpl: [128, cw] |diff| and comp planes.  Returns hd, hc for the
        max and min selectors; candidate order v=0,1,2 with strict compares
        so the first occurrence wins on exact ties.
        """
        s0 = slice(0, 2 * nj - 1, 2)
        s1 = slice(1, 2 * nj, 2)
        s2 = slice(2, 2 * nj + 1, 2)
        res = []
        for gt, ext, sel in ((ALU.is_gt, ALU.max, "M"), (ALU.is_lt, ALU.min, "m")):
            mk = m_pool.tile([BP, nj], I32, tag="mk")
            hd1 = t_pool.tile([BP, nj], F32, tag="hd1")
            hc = h_pool.tile([BP, nj], F32, tag=f"hc{tag}{sel}")
            hd = h_pool.tile([BP, nj], F32, tag=f"hd{tag}{sel}")
            nc.vector.tensor_tensor(mk[:], dpl[:, s1], dpl[:, s0], op=gt)
            nc.vector.tensor_tensor(hd1[:], dpl[:, s0], dpl[:, s1], op=ext)
            nc.scalar.copy(hc[:], cpl[:, s0])
            nc.vector.copy_predicated(hc[:], mk[:], cpl[:, s1])
            mk2 = m_pool.tile([BP, nj], I32, tag="mk")
            nc.vector.tensor_tensor(mk2[:], dpl[:, s2], hd1[:], op=gt)
            nc.vector.tensor_tensor(hd[:], hd1[:], dpl[:, s2], op=ext)
            nc.vector.copy_predicated(hc[:], mk2[:], cpl[:, s2])
            res += [hd, hc]
        return res  # hdM, hcM, hdm, hcm

    for (i0, nb) in BLOCKS:
        rr = slice(2 * i0, 2 * i0 + 2 * BP)
        AP_ = in_pool.tile([BP, 2, W], F32, tag="A")
        PP_ = in_pool.tile([BP, 2, W], F32, tag="P")
        NP_ = in_pool.tile([BP, 2, W], F32, tag="N")
        nc.sync.dma_start(AP_[:], a[rr, :].rearrange("(q t) w -> q t w", t=2))
        nc.sync.dma_start(PP_[:], p[rr, :].rearrange("(q t) w -> q t w", t=2))
        nc.sync.dma_start(NP_[:], n[rr, :].rearrange("(q t) w -> q t w", t=2))

        for ct, (j0, nj) in enumerate(JTILES):
            c0 = 2 * j0
            cw = 2 * nj + 1
            cs = slice(c0, c0 + cw)
            w = 2 * nj

            for CP_, OUT in ((PP_, outp), (NP_, outn)):
                xp = x_pool.tile([BP, 2, cw], F32, tag="xp")
                dp = dd_pool.tile([BP, 2, cw], F32, tag="dp")
                nc.gpsimd.tensor_tensor(
                    xp[:], AP_[:, :, cs], CP_[:, :, cs], op=ALU.subtract)
                nc.scalar.activation(dp[:], xp[:], ACTF.Abs)

                hdME, hcME, hdmE, hcmE = hpass(
                    dp[:, 0, :], CP_[:, 0, cs], nj, "E")
                hdMO, hcMO, hdmO, hcmO = hpass(
                    dp[:, 1, :], CP_[:, 1, cs], nj, "O")

                # shifted E-plane results (row 2i+2) via TensorE subdiag-
                # identity matmul into PSUM: out[m] = src[m+1], out[127]=0
                sh = []
                for src, stag in ((hdME, "pshdM"), (hcME, "pshcM"),
                                  (hdmE, "pshdm"), (hcmE, "pshcm")):
                    dst = ps_pool.tile([BP, nj], F32, tag=stag)
                    nc.tensor.matmul(
                        dst[:], lhsT=sm[:], rhs=src[:],
                        start=True, stop=True)
                    sh.append(dst)
                hdME1, hcME1, hdmE1, hcmE1 = sh

                # vertical merge: candidates u=0 (E0), u=1 (O), u=2 (E1)
                vcs = []
                for (hdA, hcA, hdB, hcB, hdC, hcC, gt, ext, sel) in (
                    (hdME, hcME, hdMO, hcMO, hdME1, hcME1,
                     ALU.is_gt, ALU.max, "M"),
                    (hdmE, hcmE, hdmO, hcmO, hdmE1, hcmE1,
                     ALU.is_lt, ALU.min, "m"),
                ):
                    mv = m_pool.tile([nb, nj], I32, tag="mk")
                    vd1 = t_pool.tile([nb, nj], F32, tag="hd1")
                    vc = v_pool.tile([nb, nj], F32, tag=f"vc{sel}")
                    nc.vector.tensor_tensor(
                        mv[:], hdB[:nb], hdA[:nb], op=gt)
                    nc.vector.tensor_tensor(
                        vd1[:], hdA[:nb], hdB[:nb], op=ext)
                    nc.scalar.copy(vc[:], hcA[:nb])
                    nc.vector.copy_predicated(vc[:], mv[:], hcB[:nb])
                    mv2 = m_pool.tile([nb, nj], I32, tag="mk")
                    nc.vector.tensor_tensor(mv2[:], hdC[:nb], vd1[:], op=gt)
                    nc.vector.copy_predicated(vc[:], mv2[:], hcC[:nb])
                    vcs.append(vc)
                vcM, vcm = vcs

                # row-duplicated output tile: free layout [2, w] = the two
                # output rows of each window-row; store is one linear DMA
                # with big per-partition descriptors (spreads across SDMAs)
                vv = o_pool.tile([nb, 2, w], F32, tag="vv")
                nc.vector.tensor_tensor(
                    vv[:, 0, 0:w - 1:2], vcm[:], vcM[:], op=ALU.add)
                nc.vector.tensor_tensor(
                    vv[:, 0, 1:w:2], vcm[:], vcM[:], op=ALU.add)
                nc.scalar.copy(vv[:, 1, :], vv[:, 0, :])

                base = JOFFS[ct] + 2 * i0 * w
                dst = OUT[base:base + 2 * nb * w].rearrange(
                    "(r w) -> r w", w=w)
                if ct < 2:
                    nc.gpsimd.dma_start(dst, vv[:])
                else:
                    nc.scalar.dma_start(dst, vv[:])


@with_exitstack
def _tile_kernel(ctx: ExitStack, tc, outs, ins):
    a, p, n, smat = ins
    outp, outn = outs
    _emit(ctx, tc, a, p, n, smat, outp, outn)


_CACHE = {}


def _build():
    if "nc" in _CACHE:
        return _CACHE["nc"]
    nc = bacc.Bacc(
        "TRN2",
        target_bir_lowering=False,
        debug=False,
        enable_asserts=False,
        num_devices=NCORES,
    )
    a = nc.dram_tensor("a", [SLAB, W], F32, kind="ExternalInput").ap()
    p = nc.dram_tensor("p", [SLAB, W], F32, kind="ExternalInput").ap()
    n = nc.dram_tensor("n", [SLAB, W], F32, kind="ExternalInput").ap()
    smat = nc.dram_tensor("s", [BP, BP], F32, kind="ExternalInput").ap()
    outp = nc.dram_tensor("outp", [OUT_ELEMS], F32, kind="ExternalOutput").ap()
    outn = nc.dram_tensor("outn", [OUT_ELEMS], F32, kind="ExternalOutput").ap()
    with tile.TileContext(nc) as tc:
        _tile_kernel(tc, [outp, outn], [a, p, n, smat])
    nc.compile()
    _CACHE["nc"] = nc
    return nc


def _make_in_maps(anchor, positive, negative):
    smat = np.eye(BP, k=-1, dtype=np.float32)
    in_maps = []
    for k in range(NCORES):
        r0 = OUTR * k
        m = {"s": smat}
        for name, t in (("a", anchor), ("p", positive), ("n", negative)):
            m[name] = np.ascontiguousarray(
                np.asarray(t[r0:r0 + SLAB], dtype=np.float32))
        in_maps.append(m)
    return in_maps


def _host_vrow(anchor, comp, r0):
    """Window-row at image rows r0..r0+2, all 2047 col windows; returns the
    min-sel + max-sel comp values [NJ_TOT] with exact reference semantics."""
    a3 = np.asarray(anchor[r0:r0 + 3], dtype=np.float32)
    c3 = np.asarray(comp[r0:r0 + 3], dtype=np.float32)
    d3 = np.abs(a3 - c3)
    dw = np.lib.stride_tricks.sliding_window_view(d3, 3, axis=1)[:, ::2]
    cw_ = np.lib.stride_tricks.sliding_window_view(c3, 3, axis=1)[:, ::2]
    d9 = dw.transpose(1, 0, 2).reshape(NJ_TOT, 9)
    c9 = cw_.transpose(1, 0, 2).reshape(NJ_TOT, 9)
    ar = np.arange(NJ_TOT)
    return c9[ar, np.argmin(d9, axis=1)] + c9[ar, np.argmax(d9, axis=1)]


def _assemble(results, anchor, positive, negative):
    full = {}
    for name, comp in (("outp", positive), ("outn", negative)):
        out = np.zeros((H, W), np.float32)
        for k in range(NCORES):
            flat = results[k][name]
            cols = []
            for ct, (j0, nj) in enumerate(JTILES):
                wct = 2 * nj
                cols.append(
                    flat[JOFFS[ct]:JOFFS[ct] + OUTR * wct].reshape(OUTR, wct))
            out[OUTR * k:OUTR * (k + 1), 0:2 * NJ_TOT] = np.concatenate(
                cols, axis=1)
        # host-computed window-rows: the last 2 per core (device does 254)
        for k in range(NCORES):
            for iv in (2 * VBLK, 2 * VBLK + 1):   # 254, 255
                gi = VR * k + iv
                if 2 * gi + 3 > H:
                    continue   # core 7 last row pair: overwritten below
                vals = np.repeat(_host_vrow(anchor, comp, 2 * gi), 2)
                out[2 * gi, 0:2 * NJ_TOT] = vals
                out[2 * gi + 1, 0:2 * NJ_TOT] = vals
        comp = np.asarray(comp, dtype=np.float32)
        # cols/rows H-2 replicate the last window's value a third time
        out[:, W - 2] = out[:, W - 3]
        out[H - 2, :] = out[H - 3, :]
        # uncovered last row/col keep clone semantics: min-sel + max-sel = 2c
        out[H - 1, :] = 2.0 * comp[H - 1, :]
        out[:, W - 1] = 2.0 * comp[:, W - 1]
        full[name] = out
    return full["outp"], full["outn"]


def run_on_hw(anchor, positive, negative, trace=False):
    nc = _build()
    in_maps = _make_in_maps(anchor, positive, negative)
    res = bass_utils.run_bass_kernel_spmd(
        nc, in_maps, core_ids=list(range(NCORES)), trace=trace)
    pos, neg = _assemble(res.results, anchor, positive, negative)
    return (pos, neg), res


def kernel(anchor, positive, negative):
    (pos, neg), _ = run_on_hw(anchor, positive, negative, trace=False)
    return pos, neg


# revision 13
# speedup vs baseline: 3.7554x; 1.1254x over previous
"""Trainium2 Bass kernel for nn_DCModule_25451976196444.

Sliding-window (3x3, stride 2) min/max-|anchor-comp| selection pooling:
for each window, pick the comp value where |anchor-comp| is minimal and
where it is maximal; output = sum of the two, broadcast over the window
footprint (last covering window wins).

Per core (rows sharded across 8 cores):
  - one contiguous 4 MB DMA per input per row-block loads 256 rows as
    [128, 2, 4096] "pair tiles": partition p = image rows (2p, 2p+1); the
    even/odd row planes are contiguous free-dim views
  - horizontal pass per plane merges the 3 column candidates per window
    with strict compares (exact first-occurrence ties, matching the
    row-major flattened argmax/argmin of the reference)
  - the third vertical candidate (row 2i+2) is the even-plane H-result
    shifted by one partition: done on the idle TensorE as a matmul with a
    subdiagonal identity into PSUM (no SBUF-SBUF DMA descriptor storms)
  - vertical pass merges the 3 row candidates; min+max selections are
    summed and column-duplicated on chip
  - row duplication happens in the store DMA via a step-0 source dim; the
    output DRAM layout is column-tile-major so every store is one linear
    transfer (host reassembles)
Each core computes 254 of its 256 window-rows; the host computes the last
2 window-rows per core plus the uncovered boundary rows/cols in numpy with
identical f32 semantics.
"""

import numpy as np
from contextlib import ExitStack

import concourse.bass as bass
import concourse.mybir as mybir
import concourse.tile as tile
from concourse import bacc
from concourse import bass_utils
from concourse._compat import with_exitstack

F32 = mybir.dt.float32
I32 = mybir.dt.int32
ALU = mybir.AluOpType
ACTF = mybir.ActivationFunctionType

H = 4096
W = 4096
WS = 3
ST = 2
NCORES = 8
BP = 128                    # partitions per row-block (pair tiles)
NJT = 512                   # window-cols per column tile


def _geom():
    """(Re)compute derived geometry from H/W/BP/NJT (tests patch these)."""
    global OUTR, SLAB, VR, NJ_TOT, VBLK, JTILES, JOFFS, OUT_ELEMS, BLOCKS
    OUTR = H // NCORES
    SLAB = OUTR
    VR = OUTR // 2
    NJ_TOT = (W - WS) // ST + 1
    VBLK = BP - 1
    assert VR == 2 * VBLK + 2, (VR, VBLK)
    JTILES = []
    j0 = 0
    while j0 < NJ_TOT:
        JTILES.append((j0, min(NJT, NJ_TOT - j0)))
        j0 += NJT
    JOFFS = []
    off = 0
    for (_j, _nj) in JTILES:
        JOFFS.append(off)
        off += OUTR * 2 * _nj
    OUT_ELEMS = off
    BLOCKS = [(0, VBLK), (VBLK, VBLK)]   # device window-rows 0..2*VBLK-1


_geom()


def _emit(ctx: ExitStack, tc, a, p, n, smat, outp, outn):
    nc = tc.nc

    in_pool = ctx.enter_context(tc.tile_pool(name="in", bufs=1))
    x_pool = ctx.enter_context(tc.tile_pool(name="x", bufs=2))
    dd_pool = ctx.enter_context(tc.tile_pool(name="dd", bufs=1))
    t_pool = ctx.enter_context(tc.tile_pool(name="t", bufs=3))
    m_pool = ctx.enter_context(tc.tile_pool(name="m", bufs=3))
    h_pool = ctx.enter_context(tc.tile_pool(name="h", bufs=2))
    v_pool = ctx.enter_context(tc.tile_pool(name="v", bufs=2))
    o_pool = ctx.enter_context(tc.tile_pool(name="o", bufs=2))
    c_pool = ctx.enter_context(tc.tile_pool(name="c", bufs=1))
    ps_pool = ctx.enter_context(tc.tile_pool(name="ps", bufs=1, space="PSUM"))

    sm = c_pool.tile([BP, BP], F32, tag="sm")
    nc.sync.dma_start(sm[:], smat[:])

    def hpass(dpl, cpl, nj, tag):
        """Merge the 3 column candidates of each window along the free dim.

        dpl/cpl: [128, cw] |diff| and comp planes.  Returns hd, hc for the
        max and min selectors; candidate order v=0,1,2 with strict compares
        so the first occurrence wins on exact ties.
        """
        s0 = slice(0, 2 * nj - 1, 2)
        s1 = slice(1, 2 * nj, 2)
        s2 = slice(2, 2 * nj + 1, 2)
        res = []
        for gt, ext, sel in ((ALU.is_gt, ALU.max, "M"), (ALU.is_lt, ALU.min, "m")):
            mk = m_pool.tile([BP, nj], I32, tag="mk")
            hd1 = t_pool.tile([BP, nj], F32, tag="hd1")
            hc = h_pool.tile([BP, nj], F32, tag=f"hc{tag}{sel}")
            hd = h_pool.tile([BP, nj], F32, tag=f"hd{tag}{sel}")
            nc.vector.tensor_tensor(mk[:], dpl[:, s1], dpl[:, s0], op=gt)
            nc.vector.tensor_tensor(hd1[:], dpl[:, s0], dpl[:, s1], op=ext)
            nc.scalar.copy(hc[:], cpl[:, s0])
            nc.vector.copy_predicated(hc[:], mk[:], cpl[:, s1])
            mk2 = m_pool.tile([BP, nj], I32, tag="mk")
            nc.vector.tensor_tensor(mk2[:], dpl[:, s2], hd1[:], op=gt)
            nc.vector.tensor_tensor(hd[:], hd1[:], dpl[:, s2], op=ext)
            nc.vector.copy_predicated(hc[:], mk2[:], cpl[:, s2])
            res += [hd, hc]
        return res  # hdM, hcM, hdm, hcm

    for (i0, nb) in BLOCKS:
        rr = slice(2 * i0, 2 * i0 + 2 * BP)
        AP_ = in_pool.tile([BP, 2, W], F32, tag="A")
        PP_ = in_pool.tile([BP, 2, W], F32, tag="P")
        NP_ = in_pool.tile([BP, 2, W], F32, tag="N")
        nc.sync.dma_start(AP_[:], a[rr, :].rearrange("(q t) w -> q t w", t=2))
        nc.sync.dma_start(PP_[:], p[rr, :].rearrange("(q t) w -> q t w", t=2))
        nc.sync.dma_start(NP_[:], n[rr, :].rearrange("(q t) w -> q t w", t=2))

        for ct, (j0, nj) in enumerate(JTILES):
            c0 = 2 * j0
            cw = 2 * nj + 1
            cs = slice(c0, c0 + cw)
            w = 2 * nj

            for CP_, OUT in ((PP_, outp), (NP_, outn)):
                xp = x_pool.tile([BP, 2, cw], F32, tag="xp")
                dp = dd_pool.tile([BP, 2, cw], F32, tag="dp")
                nc.gpsimd.tensor_tensor(
                    xp[:], AP_[:, :, cs], CP_[:, :, cs], op=ALU.subtract)
                nc.scalar.activation(dp[:], xp[:], ACTF.Abs)

                hdME, hcME, hdmE, hcmE = hpass(
                    dp[:, 0, :], CP_[:, 0, cs], nj, "E")
                hdMO, hcMO, hdmO, hcmO = hpass(
                    dp[:, 1, :], CP_[:, 1, cs], nj, "O")

                # shifted E-plane results (row 2i+2) via TensorE subdiag-
                # identity matmul into PSUM: out[m] = src[m+1], out[127]=0
                sh = []
                for src, stag in ((hdME, "pshdM"), (hcME, "pshcM"),
                                  (hdmE, "pshdm"), (hcmE, "pshcm")):
                    dst = ps_pool.tile([BP, nj], F32, tag=stag)
                    nc.tensor.matmul(
                        dst[:], lhsT=sm[:], rhs=src[:],
                        start=True, stop=True)
                    sh.append(dst)
                hdME1, hcME1, hdmE1, hcmE1 = sh

                # vertical merge: candidates u=0 (E0), u=1 (O), u=2 (E1)
                vcs = []
                for (hdA, hcA, hdB, hcB, hdC, hcC, gt, ext, sel) in (
                    (hdME, hcME, hdMO, hcMO, hdME1, hcME1,
                     ALU.is_gt, ALU.max, "M"),
                    (hdmE, hcmE, hdmO, hcmO, hdmE1, hcmE1,
                     ALU.is_lt, ALU.min, "m"),
                ):
                    mv = m_pool.tile([nb, nj], I32, tag="mk")
                    vd1 = t_pool.tile([nb, nj], F32, tag="hd1")
                    vc = v_pool.tile([nb, nj], F32, tag=f"vc{sel}")
                    nc.vector.tensor_tensor(
                        mv[:], hdB[:nb], hdA[:nb], op=gt)
                    nc.vector.tensor_tensor(
                        vd1[:], hdA[:nb], hdB[:nb], op=ext)
                    nc.scalar.copy(vc[:], hcA[:nb])
                    nc.vector.copy_predicated(vc[:], mv[:], hcB[:nb])
                    mv2 = m_pool.tile([nb, nj], I32, tag="mk")
                    nc.vector.tensor_tensor(mv2[:], hdC[:nb], vd1[:], op=gt)
                    nc.vector.copy_predicated(vc[:], mv2[:], hcC[:nb])
                    vcs.append(vc)
                vcM, vcm = vcs

                # row-duplicated output tile: free layout [2, w] = the two
                # output rows of each window-row; store is one linear DMA
                # with big per-partition descriptors (spreads across SDMAs)
                vv = o_pool.tile([nb, 2, w], F32, tag="vv")
                nc.vector.tensor_tensor(
                    vv[:, 0, 0:w - 1:2], vcm[:], vcM[:], op=ALU.add)
                nc.vector.tensor_tensor(
                    vv[:, 0, 1:w:2], vcm[:], vcM[:], op=ALU.add)
                nc.scalar.copy(vv[:, 1, :], vv[:, 0, :])

                base = JOFFS[ct] + 2 * i0 * w
                dst = OUT[base:base + 2 * nb * w].rearrange(
                    "(r w) -> r w", w=w)
                nc.gpsimd.dma_start(dst, vv[:])


@with_exitstack
def _tile_kernel(ctx: ExitStack, tc, outs, ins):
    a, p, n, smat = ins
    outp, outn = outs
    _emit(ctx, tc, a, p, n, smat, outp, outn)


_CACHE = {}


def _build():
    if "nc" in _CACHE:
        return _CACHE["nc"]
    nc = bacc.Bacc(
        "TRN2",
        target_bir_lowering=False,
        debug=False,
        enable_asserts=False,
        num_devices=NCORES,
    )
    a = nc.dram_tensor("a", [SLAB, W], F32, kind="ExternalInput").ap()
    p = nc.dram_tensor("p", [SLAB, W], F32, kind="ExternalInput").ap()
    n = nc.dram_tensor("n", [SLAB, W], F32, kind="ExternalInput").ap()
    smat = nc.dram_tensor("s", [BP, BP], F32, kind="ExternalInput").ap()
    outp = nc.dram_tensor("outp", [OUT_ELEMS], F32, kind="ExternalOutput").ap()
    outn = nc.dram_tensor("outn", [OUT_ELEMS], F32, kind="ExternalOutput").ap()
    with tile.TileContext(nc) as tc:
        _tile_kernel(tc, [outp, outn], [a, p, n, smat])
    nc.compile()
    _CACHE["nc"] = nc
    return nc


def _make_in_maps(anchor, positive, negative):
    smat = np.eye(BP, k=-1, dtype=np.float32)
    in_maps = []
    for k in range(NCORES):
        r0 = OUTR * k
        m = {"s": smat}
        for name, t in (("a", anchor), ("p", positive), ("n", negative)):
            m[name] = np.ascontiguousarray(
                np.asarray(t[r0:r0 + SLAB], dtype=np.float32))
        in_maps.append(m)
    return in_maps


def _host_vrow(anchor, comp, r0):
    """Window-row at image rows r0..r0+2, all 2047 col windows; returns the
    min-sel + max-sel comp values [NJ_TOT] with exact reference semantics."""
    a3 = np.asarray(anchor[r0:r0 + 3], dtype=np.float32)
    c3 = np.asarray(comp[r0:r0 + 3], dtype=np.float32)
    d3 = np.abs(a3 - c3)
    dw = np.lib.stride_tricks.sliding_window_view(d3, 3, axis=1)[:, ::2]
    cw_ = np.lib.stride_tricks.sliding_window_view(c3, 3, axis=1)[:, ::2]
    d9 = dw.transpose(1, 0, 2).reshape(NJ_TOT, 9)
    c9 = cw_.transpose(1, 0, 2).reshape(NJ_TOT, 9)
    ar = np.arange(NJ_TOT)
    return c9[ar, np.argmin(d9, axis=1)] + c9[ar, np.argmax(d9, axis=1)]


def _assemble(results, anchor, positive, negative):
    full = {}
    for name, comp in (("outp", positive), ("outn", negative)):
        out = np.zeros((H, W), np.float32)
        for k in range(NCORES):
            flat = results[k][name]
            cols = []
            for ct, (j0, nj) in enumerate(JTILES):
                wct = 2 * nj
                cols.append(
                    flat[JOFFS[ct]:JOFFS[ct] + OUTR * wct].reshape(OUTR, wct))
            out[OUTR * k:OUTR * (k + 1), 0:2 * NJ_TOT] = np.concatenate(
                cols, axis=1)
        # host-computed window-rows: the last 2 per core (device does 254)
        for k in range(NCORES):
            for iv in (2 * VBLK, 2 * VBLK + 1):   # 254, 255
                gi = VR * k + iv
                if 2 * gi + 3 > H:
                    continue   # core 7 last row pair: overwritten below
                vals = np.repeat(_host_vrow(anchor, comp, 2 * gi), 2)
                out[2 * gi, 0:2 * NJ_TOT] = vals
                out[2 * gi + 1, 0:2 * NJ_TOT] = vals
        comp = np.asarray(comp, dtype=np.float32)
        # cols/rows H-2 replicate the last window's value a third time
        out[:, W - 2] = out[:, W - 3]
        out[H - 2, :] = out[H - 3, :]
        # uncovered last row/col keep clone semantics: min-sel + max-sel = 2c
        out[H - 1, :] = 2.0 * comp[H - 1, :]
        out[:, W - 1] = 2.0 * comp[:, W - 1]
        full[name] = out
    return full["outp"], full["outn"]


def run_on_hw(anchor, positive, negative, trace=False):
    nc = _build()
    in_maps = _make_in_maps(anchor, positive, negative)
    res = bass_utils.run_bass_kernel_spmd(
        nc, in_maps, core_ids=list(range(NCORES)), trace=trace)
    pos, neg = _assemble(res.results, anchor, positive, negative)
    return (pos, neg), res


def kernel(anchor, positive, negative):
    (pos, neg), _ = run_on_hw(anchor, positive, negative, trace=False)
    return pos, neg


# revision 18
# speedup vs baseline: 3.8896x; 1.0357x over previous
"""Trainium2 Bass kernel for nn_DCModule_25451976196444.

Sliding-window (3x3, stride 2) min/max-|anchor-comp| selection pooling:
for each window, pick the comp value where |anchor-comp| is minimal and
where it is maximal; output = sum of the two, broadcast over the window
footprint (last covering window wins).

Per core (rows sharded across 8 cores):
  - one contiguous 4 MB DMA per input per row-block loads 256 rows as
    [128, 2, 4096] "pair tiles": partition p = image rows (2p, 2p+1); the
    even/odd row planes are contiguous free-dim views
  - horizontal pass per plane merges the 3 column candidates per window
    with strict compares (exact first-occurrence ties, matching the
    row-major flattened argmax/argmin of the reference)
  - the third vertical candidate (row 2i+2) is the even-plane H-result
    shifted by one partition: done on the idle TensorE as a matmul with a
    subdiagonal identity into PSUM (no SBUF-SBUF DMA descriptor storms)
  - vertical pass merges the 3 row candidates; min+max selections are
    summed and column-duplicated on chip
  - row duplication happens in the store DMA via a step-0 source dim; the
    output DRAM layout is column-tile-major so every store is one linear
    transfer (host reassembles)
Each core computes 254 of its 256 window-rows; the host computes the last
2 window-rows per core plus the uncovered boundary rows/cols in numpy with
identical f32 semantics.
"""

import numpy as np
from contextlib import ExitStack

import concourse.bass as bass
import concourse.mybir as mybir
import concourse.tile as tile
from concourse import bacc
from concourse import bass_utils
from concourse._compat import with_exitstack

F32 = mybir.dt.float32
I32 = mybir.dt.int32
U8 = mybir.dt.uint8
ALU = mybir.AluOpType
ACTF = mybir.ActivationFunctionType

H = 4096
W = 4096
WS = 3
ST = 2
NCORES = 8
BP = 128                    # partitions per row-block (pair tiles)
NJT = 512                   # window-cols per column tile


def _geom():
    """(Re)compute derived geometry from H/W/BP/NJT (tests patch these)."""
    global OUTR, SLAB, VR, NJ_TOT, VBLK, JTILES, JOFFS, OUT_ELEMS, BLOCKS
    OUTR = H // NCORES
    SLAB = OUTR
    VR = OUTR // 2
    NJ_TOT = (W - WS) // ST + 1
    VBLK = BP - 1
    assert VR == 2 * VBLK + 2, (VR, VBLK)
    JTILES = []
    j0 = 0
    while j0 < NJ_TOT:
        JTILES.append((j0, min(NJT, NJ_TOT - j0)))
        j0 += NJT
    JOFFS = []
    off = 0
    for (_j, _nj) in JTILES:
        JOFFS.append(off)
        off += OUTR * 2 * _nj
    OUT_ELEMS = off
    BLOCKS = [(0, VBLK), (VBLK, VBLK)]   # device window-rows 0..2*VBLK-1


_geom()


def _emit(ctx: ExitStack, tc, a, p, n, smat, outp, outn):
    nc = tc.nc

    in_pool = ctx.enter_context(tc.tile_pool(name="in", bufs=1))
    x_pool = ctx.enter_context(tc.tile_pool(name="x", bufs=2))
    dd_pool = ctx.enter_context(tc.tile_pool(name="dd", bufs=1))
    t_pool = ctx.enter_context(tc.tile_pool(name="t", bufs=3))
    m_pool = ctx.enter_context(tc.tile_pool(name="m", bufs=3))
    h_pool = ctx.enter_context(tc.tile_pool(name="h", bufs=2))
    v_pool = ctx.enter_context(tc.tile_pool(name="v", bufs=2))
    o_pool = ctx.enter_context(tc.tile_pool(name="o", bufs=2))
    c_pool = ctx.enter_context(tc.tile_pool(name="c", bufs=1))
    ps_pool = ctx.enter_context(tc.tile_pool(name="ps", bufs=1, space="PSUM"))

    sm = c_pool.tile([BP, BP], F32, tag="sm")
    nc.sync.dma_start(sm[:], smat[:])

    def hpass(dpl, cpl, nj):
        """Merge the 3 column candidates of each window, batched over both
        row planes: dpl/cpl are [BP, 2, cw] |diff| and comp pair views.
        Returns padded hd, hc tiles [BP, 2, nj(+1)] for max and min
        selectors; candidate order v=0,1,2 with strict compares so the
        first occurrence wins on exact ties.
        """
        s0 = slice(0, 2 * nj - 1, 2)
        s1 = slice(1, 2 * nj, 2)
        s2 = slice(2, 2 * nj + 1, 2)
        res = []
        for gt, ext, sel in ((ALU.is_gt, ALU.max, "M"), (ALU.is_lt, ALU.min, "m")):
            mk_t = m_pool.tile([BP, 2, nj + 1], U8, tag="mk")
            mk = mk_t[:, :, 0:nj]
            hd1 = t_pool.tile([BP, 2, nj], F32, tag="hd1")
            hc_t = h_pool.tile([BP, 2, nj + 1], F32, tag=f"hc{sel}")
            hc = hc_t[:, :, 0:nj]
            hd = h_pool.tile([BP, 2, nj], F32, tag=f"hd{sel}")
            nc.vector.tensor_tensor(mk, dpl[:, :, s1], dpl[:, :, s0], op=gt)
            nc.vector.tensor_tensor(hd1[:], dpl[:, :, s0], dpl[:, :, s1], op=ext)
            nc.scalar.copy(hc, cpl[:, :, s0])
            nc.vector.copy_predicated(hc, mk, cpl[:, :, s1])
            mk2_t = m_pool.tile([BP, 2, nj + 1], U8, tag="mk")
            mk2 = mk2_t[:, :, 0:nj]
            nc.vector.tensor_tensor(mk2, dpl[:, :, s2], hd1[:], op=gt)
            nc.vector.tensor_tensor(hd[:], hd1[:], dpl[:, :, s2], op=ext)
            nc.vector.copy_predicated(hc, mk2, cpl[:, :, s2])
            res += [hd, hc_t]
        return res  # hdM, hcM(padded), hdm, hcm(padded)

    for (i0, nb) in BLOCKS:
        rr = slice(2 * i0, 2 * i0 + 2 * BP)
        AP_ = in_pool.tile([BP, 2, W], F32, tag="A")
        PP_ = in_pool.tile([BP, 2, W], F32, tag="P")
        NP_ = in_pool.tile([BP, 2, W], F32, tag="N")
        nc.sync.dma_start(AP_[:], a[rr, :].rearrange("(q t) w -> q t w", t=2))
        nc.sync.dma_start(PP_[:], p[rr, :].rearrange("(q t) w -> q t w", t=2))
        nc.sync.dma_start(NP_[:], n[rr, :].rearrange("(q t) w -> q t w", t=2))

        for ct, (j0, nj) in enumerate(JTILES):
            c0 = 2 * j0
            cw = 2 * nj + 1
            cs = slice(c0, c0 + cw)
            w = 2 * nj

            for CP_, OUT in ((PP_, outp), (NP_, outn)):
                xp = x_pool.tile([BP, 2, cw], F32, tag="xp")
                dp = dd_pool.tile([BP, 2, cw], F32, tag="dp")
                nc.gpsimd.tensor_tensor(
                    xp[:], AP_[:, :, cs], CP_[:, :, cs], op=ALU.subtract)
                nc.scalar.activation(dp[:], xp[:], ACTF.Abs)

                hdM, hcM, hdm, hcm = hpass(dp, CP_[:, :, cs], nj)

                # shifted E-plane results (row 2i+2) via TensorE subdiag-
                # identity matmul into PSUM: out[m] = src[m+1], out[127]=0
                sh = []
                for srct, stag in ((hdM, "pshdM"), (hcM, "pshcM"),
                                   (hdm, "pshdm"), (hcm, "pshcm")):
                    dst = ps_pool.tile([BP, nj], F32, tag=stag)
                    nc.tensor.matmul(
                        dst[:], lhsT=sm[:], rhs=srct[:, 0, 0:nj],
                        start=True, stop=True)
                    sh.append(dst)
                hdME1, hcME1, hdmE1, hcmE1 = sh

                # vertical merge: candidates u=0 (E0), u=1 (O), u=2 (E1)
                vcs = []
                for (hh, cc, hdC, hcC, gt, ext, sel) in (
                    (hdM, hcM, hdME1, hcME1, ALU.is_gt, ALU.max, "M"),
                    (hdm, hcm, hdmE1, hcmE1, ALU.is_lt, ALU.min, "m"),
                ):
                    mv = m_pool.tile([nb, nj], U8, tag="mk")
                    vd1 = t_pool.tile([nb, nj], F32, tag="hd1")
                    vc = v_pool.tile([nb, nj], F32, tag=f"vc{sel}")
                    nc.vector.tensor_tensor(
                        mv[:], hh[:nb, 1], hh[:nb, 0], op=gt)
                    nc.vector.tensor_tensor(
                        vd1[:], hh[:nb, 0], hh[:nb, 1], op=ext)
                    nc.scalar.copy(vc[:], cc[:nb, 0, 0:nj])
                    nc.vector.copy_predicated(vc[:], mv[:], cc[:nb, 1, 0:nj])
                    mv2 = m_pool.tile([nb, nj], U8, tag="mk")
                    nc.vector.tensor_tensor(mv2[:], hdC[:nb], vd1[:], op=gt)
                    nc.vector.copy_predicated(vc[:], mv2[:], hcC[:nb])
                    vcs.append(vc)
                vcM, vcm = vcs

                # row-duplicated output tile: free layout [2, w] = the two
                # output rows of each window-row; store is one linear DMA
                # with big per-partition descriptors (spreads across SDMAs)
                vv = o_pool.tile([nb, 2, w], F32, tag="vv")
                nc.vector.tensor_tensor(
                    vv[:, 0, 0:w - 1:2], vcm[:], vcM[:], op=ALU.add)
                nc.vector.tensor_tensor(
                    vv[:, 0, 1:w:2], vcm[:], vcM[:], op=ALU.add)
                nc.scalar.copy(vv[:, 1, :], vv[:, 0, :])

                base = JOFFS[ct] + 2 * i0 * w
                dst = OUT[base:base + 2 * nb * w].rearrange(
                    "(r w) -> r w", w=w)
                nc.gpsimd.dma_start(dst, vv[:])


@with_exitstack
def _tile_kernel(ctx: ExitStack, tc, outs, ins):
    a, p, n, smat = ins
    outp, outn = outs
    _emit(ctx, tc, a, p, n, smat, outp, outn)


_CACHE = {}


def _build():
    if "nc" in _CACHE:
        return _CACHE["nc"]
    nc = bacc.Bacc(
        "TRN2",
        target_bir_lowering=False,
        debug=False,
        enable_asserts=False,
        num_devices=NCORES,
    )
    a = nc.dram_tensor("a", [SLAB, W], F32, kind="ExternalInput").ap()
    p = nc.dram_tensor("p", [SLAB, W], F32, kind="ExternalInput").ap()
    n = nc.dram_tensor("n", [SLAB, W], F32, kind="ExternalInput").ap()
    smat = nc.dram_tensor("s", [BP, BP], F32, kind="ExternalInput").ap()
    outp = nc.dram_tensor("outp", [OUT_ELEMS], F32, kind="ExternalOutput").ap()
    outn = nc.dram_tensor("outn", [OUT_ELEMS], F32, kind="ExternalOutput").ap()
    with tile.TileContext(nc) as tc:
        _tile_kernel(tc, [outp, outn], [a, p, n, smat])
    nc.compile()
    _CACHE["nc"] = nc
    return nc


def _make_in_maps(anchor, positive, negative):
    smat = np.eye(BP, k=-1, dtype=np.float32)
    in_maps = []
    for k in range(NCORES):
        r0 = OUTR * k
        m = {"s": smat}
        for name, t in (("a", anchor), ("p", positive), ("n", negative)):
            m[name] = np.ascontiguousarray(
                np.asarray(t[r0:r0 + SLAB], dtype=np.float32))
        in_maps.append(m)
    return in_maps


def _host_vrow(anchor, comp, r0):
    """Window-row at image rows r0..r0+2, all 2047 col windows; returns the
    min-sel + max-sel comp values [NJ_TOT] with exact reference semantics."""
    a3 = np.asarray(anchor[r0:r0 + 3], dtype=np.float32)
    c3 = np.asarray(comp[r0:r0 + 3], dtype=np.float32)
    d3 = np.abs(a3 - c3)
    dw = np.lib.stride_tricks.sliding_window_view(d3, 3, axis=1)[:, ::2]
    cw_ = np.lib.stride_tricks.sliding_window_view(c3, 3, axis=1)[:, ::2]
    d9 = dw.transpose(1, 0, 2).reshape(NJ_TOT, 9)
    c9 = cw_.transpose(1, 0, 2).reshape(NJ_TOT, 9)
    ar = np.arange(NJ_TOT)
    return c9[ar, np.argmin(d9, axis=1)] + c9[ar, np.argmax(d9, axis=1)]


def _assemble(results, anchor, positive, negative):
    full = {}
    for name, comp in (("outp", positive), ("outn", negative)):
        out = np.zeros((H, W), np.float32)
        for k in range(NCORES):
            flat = results[k][name]
            cols = []
            for ct, (j0, nj) in enumerate(JTILES):
                wct = 2 * nj
                cols.append(
                    flat[JOFFS[ct]:JOFFS[ct] + OUTR * wct].reshape(OUTR, wct))
            out[OUTR * k:OUTR * (k + 1), 0:2 * NJ_TOT] = np.concatenate(
                cols, axis=1)
        # host-computed window-rows: the last 2 per core (device does 254)
        for k in range(NCORES):
            for iv in (2 * VBLK, 2 * VBLK + 1):   # 254, 255
                gi = VR * k + iv
                if 2 * gi + 3 > H:
                    continue   # core 7 last row pair: overwritten below
                vals = np.repeat(_host_vrow(anchor, comp, 2 * gi), 2)
                out[2 * gi, 0:2 * NJ_TOT] = vals
                out[2 * gi + 1, 0:2 * NJ_TOT] = vals
        comp = np.asarray(comp, dtype=np.float32)
        # cols/rows H-2 replicate the last window's value a third time
        out[:, W - 2] = out[:, W - 3]
        out[H - 2, :] = out[H - 3, :]
        # uncovered last row/col keep clone semantics: min-sel + max-sel = 2c
        out[H - 1, :] = 2.0 * comp[H - 1, :]
        out[:, W - 1] = 2.0 * comp[:, W - 1]
        full[name] = out
    return full["outp"], full["outn"]


def run_on_hw(anchor, positive, negative, trace=False):
    nc = _build()
    in_maps = _make_in_maps(anchor, positive, negative)
    res = bass_utils.run_bass_kernel_spmd(
        nc, in_maps, core_ids=list(range(NCORES)), trace=trace)
    pos, neg = _assemble(res.results, anchor, positive, negative)
    return (pos, neg), res


def kernel(anchor, positive, negative):
    (pos, neg), _ = run_on_hw(anchor, positive, negative, trace=False)
    return pos, neg
